# revision 27
# baseline (speedup 1.0000x reference)
"""3-layer GAT (2 heads x 128) on 8 TRN2 NeuronCores — Bass/Tile kernel, v2.

Sharding: nodes partitioned across cores by destination (graph parallel);
weights replicated; per-layer AllGather of transposed features.

v2 design (vs v1): the per-edge work is Q7-descriptor-bound (~6ns/desc), so
descriptors are minimized:
  - edge slots are laid out with partition = dst slot (dst's position within
    its 128-node block), so ed[dst] is a per-partition broadcast — the
    per-edge ed gather (1/3 of all descriptors in v1) is gone.
  - aggregation over edge slots = identity-matmul PSUM accumulation (sum
    over the free/slot dim), so the one-hot S tiles (+32MB/layer DMA) are
    gone.
  - buckets are exact-sized per block (graph known at compile time); pad
    slots (dst-degree imbalance) gather row 0 and are masked.
  - self-loops ride in-bucket as slot 0 of each partition.
  - sources are split into two overlapping table halves (int16 gather idx
    limit); overlap-band edges balance the two buckets per partition.

Per layer: esed (ed per own node, from resident hT) -> phase A (replicated:
full table h@W_ext -> DRAM, 260 cols) -> phase B per dst block: 2 gathers
(768B/edge), exm = mask*exp(lrelu(es+ed)), msg = [xh*exm | exm], psum +=
I @ msg[t] over slots; epilogue: head-mean/denominator, bias, ELU,
PE-transpose into next layer's hT. AllGather hT between layers.
"""
import dataclasses
import numpy as np

import concourse.bass as bass
import concourse.bacc as bacc
import concourse.mybir as mybir
import concourse.tile as tile

f32 = mybir.dt.float32
f32r = mybir.dt.bfloat16
i16 = mybir.dt.int16
ALU = mybir.AluOpType
ACTF = mybir.ActivationFunctionType

ROW = 384          # table row stride (elem_size for gather; 768B)
TCOL = 260         # used table cols: 256 xh + 2 es + 2 ed
HALF = 32768       # rows per gather window (int16 idx limit)
NWIN = 3           # overlapping source windows (balance buckets)
DUP = 17408        # rows [0, DUP) are duplicated after the main table so
                   # window 2 can reach them (near-full 2-window coverage)


@dataclasses.dataclass(frozen=True)
class Cfg:
    n: int = 50000
    ncores: int = 8
    nlayers: int = 3
    hid: int = 128

    @property
    def nb(self):  return self.n // self.ncores
    @property
    def cpb(self):  return (self.nb + 127) // 128
    @property
    def npc(self):  return self.cpb * 128
    @property
    def npad(self): return self.ncores * self.npc
    @property
    def wbase(self):
        # window start rows over [main | dup of rows 0:DUP]
        return [0, DUP, 2 * DUP]


# ---------------------------------------------------------------- host side

def pack_nodes(cfg, deg):
    """perm [N] -> slot. Global degree-desc sort dealt round-robin to cores,
    so every core's block b holds nodes of near-identical degree (the
    per-(block,window) bucket size T is a cross-core max)."""
    order = np.argsort(-deg, kind="stable")
    perm = np.full(cfg.n, -1, dtype=np.int64)
    i = np.arange(cfg.n)
    perm[order] = (i % cfg.ncores) * cfg.npc + i // cfg.ncores
    return perm


def preprocess(cfg, edge_index):
    """Build per-core gather idx + mask arrays and global per-block bucket
    sizes (T must be identical across cores: SPMD single program)."""
    src0 = np.asarray(edge_index[0], dtype=np.int64)
    dst0 = np.asarray(edge_index[1], dtype=np.int64)
    deg = np.bincount(dst0, minlength=cfg.n) + 1     # incl self-loop
    perm = pack_nodes(cfg, deg)
    wbase = cfg.wbase

    ps = perm[src0]
    pd = perm[dst0]

    # per-slot edge lists: sort edges by dst slot
    order = np.argsort(pd, kind="stable")
    ps_s, pd_s = ps[order], pd[order]
    starts = np.searchsorted(pd_s, np.arange(cfg.npad + 1))

    inv = np.empty(cfg.npad, dtype=np.int64)   # slot -> node id (or -1)
    inv.fill(-1)
    inv[perm] = np.arange(cfg.n)

    # bucket rows per (core, block, partition, window); greedy balance of
    # flexible rows (windows overlap) to minimize per-window maxima
    nW = np.zeros((NWIN, cfg.ncores, cfg.cpb, 128), dtype=np.int32)
    lists = {}
    for c in range(cfg.ncores):
        for b in range(cfg.cpb):
            for p in range(128):
                slot = c * cfg.npc + b * 128 + p
                if inv[slot] < 0:
                    continue
                # self-loops are folded on-chip (not gathered)
                rows = list(ps_s[starts[slot]:starts[slot + 1]])
                lw = [[] for _ in range(NWIN)]
                flex = []
                for r in rows:
                    # positions of row r: r (main) and npad+r (dup, r < DUP)
                    elig = [w for w in range(NWIN)
                            if wbase[w] <= r < wbase[w] + HALF
                            or (r < DUP
                                and wbase[w] <= cfg.npad + r < wbase[w] + HALF)]
                    if len(elig) == 1:
                        lw[elig[0]].append(r)
                    else:
                        flex.append((r, elig))
                for r, elig in flex:
                    w = min(elig, key=lambda w: len(lw[w]))
                    lw[w].append(r)
                for w in range(NWIN):
                    lists[(w, c, b, p)] = lw[w]
                    nW[w, c, b, p] = len(lw[w])

    # global per-(block, window) T (max across cores & partitions)
    TW = nW.max(axis=(1, 3)).astype(np.int64)    # [NWIN, cpb]

    sumT = int(TW.sum())
    idx_flat = np.zeros((cfg.ncores, sumT * 128), dtype=np.int16)
    mask = np.zeros((cfg.ncores, 128, sumT, 2), dtype=np.float32)
    seg_off = []   # per (b): slot offset of block segment start
    off = 0
    for b in range(cfg.cpb):
        seg_off.append(off)
        off += int(TW[:, b].sum())
    for c in range(cfg.ncores):
        for b in range(cfg.cpb):
            o = seg_off[b]
            for w in range(NWIN):
                tw = int(TW[w, b])
                for p in range(128):
                    for t, r in enumerate(lists.get((w, c, b, p), [])):
                        pos = r
                        if not (wbase[w] <= pos < wbase[w] + HALF):
                            pos = cfg.npad + r    # dup copy
                        idx_flat[c, (o + t) * 128 + p] = pos - wbase[w]
                        mask[c, p, o + t, :] = 1.0
                o += tw
    return dict(perm=perm, TW=TW, seg_off=seg_off, sumT=sumT,
                idx_flat=idx_flat, mask=mask)


def wrap_rep(idx):
    """[K] int16 -> dma_gather wrapped layout [128, K/16]."""
    K = idx.shape[-1]
    w = idx.reshape(K // 16, 16).T.copy()       # [16, K/16]
    return np.tile(w, (8, 1)).copy()


def host_arrays(cfg, x, edge_index, params):
    import ml_dtypes
    bfl = ml_dtypes.bfloat16
    pp = preprocess(cfg, edge_index)
    perm = pp["perm"]

    xpad = np.zeros((cfg.npad, 128), dtype=np.float32)
    xpad[perm] = np.asarray(x, np.float32)
    xT_stack = np.ascontiguousarray(
        xpad.reshape(cfg.ncores, cfg.npc, 128).transpose(0, 2, 1)
        .reshape(cfg.ncores * 128, cfg.npc))

    w_ext = np.zeros((cfg.nlayers, 128, TCOL), dtype=np.float32)
    bias = np.zeros((cfg.nlayers, 128, 128), dtype=np.float32)
    for li, (W, a_s, a_d, b) in enumerate(params):
        W = np.asarray(W, np.float32)
        w_ext[li, :, :256] = W
        w_ext[li, :, 256] = W[:, :128] @ np.asarray(a_s, np.float32)[0]
        w_ext[li, :, 257] = W[:, 128:] @ np.asarray(a_s, np.float32)[1]
        w_ext[li, :, 258] = W[:, :128] @ np.asarray(a_d, np.float32)[0]
        w_ext[li, :, 259] = W[:, 128:] @ np.asarray(a_d, np.float32)[1]
        bias[li] = np.tile(np.asarray(b, np.float32)[None, :], (128, 1))

    # wrapped idx: concat per-(b) segments (each segment len 128*(TA+TB))
    per_core = []
    for c in range(cfg.ncores):
        idxw = wrap_rep(pp["idx_flat"][c])      # [128, sumT*8]
        per_core.append(dict(
            xT_stack=xT_stack.astype(bfl),
            xT_local=np.ascontiguousarray(
                xT_stack[c * 128:(c + 1) * 128]).astype(bfl),
            w_ext=w_ext.astype(bfl), bias=bias,
            ident=np.eye(128, dtype=np.float32),
            identb=np.eye(128, dtype=np.float32).astype(bfl),
            idxw=idxw,
            maskw=np.ascontiguousarray(
                pp["mask"][c].reshape(128, pp["sumT"] * 2)),
        ))
    return pp, per_core


# -------------------------------------------------------------- device side

def build_nc(cfg, pp):
    nc = bacc.Bacc("TRN2", num_devices=cfg.ncores, num_swdge_queues=4)
    NPC, CPB, NL, NSH = cfg.npc, cfg.cpb, cfg.nlayers, cfg.ncores
    TW, seg_off, sumT = pp["TW"], pp["seg_off"], pp["sumT"]
    TTCAP = int(TW.sum(axis=0).max())
    NROWS = NSH * NPC + DUP       # main table + duplicated low rows
    SPLIT = 32 * 128

    xT_stack = nc.dram_tensor("xT_stack", [NSH * 128, NPC], f32r, kind="ExternalInput")
    xT_local = nc.dram_tensor("xT_local", [128, NPC], f32r, kind="ExternalInput")
    w_ext_in = nc.dram_tensor("w_ext", [NL, 128, TCOL], f32r, kind="ExternalInput")
    bias_in = nc.dram_tensor("bias", [NL, 128, 128], f32, kind="ExternalInput")
    ident_in = nc.dram_tensor("ident", [128, 128], f32, kind="ExternalInput")
    identb_in = nc.dram_tensor("identb", [128, 128], f32r, kind="ExternalInput")
    idx_in = nc.dram_tensor("idxw", [128, sumT * 8], i16, kind="ExternalInput")
    mask_in = nc.dram_tensor("maskw", [128, sumT * 2], f32, kind="ExternalInput")
    out = nc.dram_tensor("out", [NPC, 128], f32, kind="ExternalOutput")

    with tile.TileContext(nc) as tc:
        with (
            tc.tile_pool(name="const", bufs=1) as constp,
            tc.tile_pool(name="dram", bufs=2, space="DRAM") as dramp,
            tc.tile_pool(name="hT", bufs=1) as hTp,
            tc.tile_pool(name="esed", bufs=1) as esedp,
            tc.tile_pool(name="slabA", bufs=2) as slabAp,
            tc.tile_pool(name="rowA", bufs=6) as rowAp,
            tc.tile_pool(name="g1", bufs=2) as g1p,
            tc.tile_pool(name="att", bufs=4) as attp,
            tc.tile_pool(name="ep", bufs=3) as epp,
            tc.tile_pool(name="psumA", bufs=3, space="PSUM") as psumAp,
            tc.tile_pool(name="psumB", bufs=3, space="PSUM") as psumBp,
            tc.tile_pool(name="psumT", bufs=1, space="PSUM") as psumTp,
        ):
            idx_sb = constp.tile([128, sumT * 8], i16)
            nc.sync.dma_start(idx_sb[:], idx_in.ap())
            mask_sb = constp.tile([128, sumT, 2], f32)
            nc.sync.dma_start(mask_sb[:], mask_in.ap())
            w_sb = constp.tile([128, NL * TCOL], f32r)
            bias_sb = constp.tile([128, NL * 128], f32)
            for li in range(NL):
                nc.sync.dma_start(w_sb[:, li * TCOL:(li + 1) * TCOL], w_ext_in.ap()[li])
                nc.sync.dma_start(bias_sb[:, li * 128:(li + 1) * 128], bias_in.ap()[li])
            ident_sb = constp.tile([128, 128], f32)
            nc.sync.dma_start(ident_sb[:], ident_in.ap())
            identb_sb = constp.tile([128, 128], f32r)
            nc.sync.dma_start(identb_sb[:], identb_in.ap())

            # gather count registers (one per distinct 128*T)
            regs = {}
            for b in range(CPB):
                for w in range(NWIN):
                    T = int(TW[w, b])
                    if T and T not in regs:
                        regs[T] = nc.gpsimd.to_reg(128 * T)

            # resident own-transposed-h + own table rows: double buffered
            hT_buf = [hTp.tile([128, NPC], f32r, name=f"hT{i}") for i in range(2)]
            nc.sync.dma_start(hT_buf[0][:], xT_local.ap())
            xho_buf = [esedp.tile([128, CPB, TCOL], f32r, name=f"xho{i}")
                       for i in range(2)]
            JSPLIT = SPLIT // 128

            def w_of(li):
                return w_sb[:, li * TCOL:(li + 1) * TCOL]

            def emit_xh_own_j(li, j, hsrc):
                """own-node table row (xh|es|ed) for layer li, block j."""
                psA = psumAp.tile([128, TCOL], f32)
                nc.tensor.matmul(
                    psA[:], hsrc[:, j * 128:(j + 1) * 128],
                    w_of(li)[:, :TCOL], start=True, stop=True)
                if j % 2:
                    nc.scalar.activation(xho_buf[li % 2][:, j, :], psA[:],
                                         ACTF.Copy)
                else:
                    nc.vector.tensor_copy(xho_buf[li % 2][:, j, :], psA[:])

            def a_block(li, table, hTs, col0, s, j):
                psA = psumAp.tile([128, TCOL], f32)
                nc.tensor.matmul(
                    psA[:], hTs[:, j * 128 - col0:(j + 1) * 128 - col0],
                    w_of(li)[:, :TCOL], start=True, stop=True)
                tA = rowAp.tile([128, TCOL], f32r)
                if j % 2:
                    nc.scalar.activation(tA[:], psA[:], ACTF.Copy)
                else:
                    nc.vector.tensor_copy(tA[:], psA[:])
                base = s * NPC + j * 128
                nc.sync.dma_start(table[base:base + 128, 0:TCOL], tA[:])
                if base < DUP:
                    nc.sync.dma_start(
                        table[NSH * NPC + base:NSH * NPC + base + 128,
                              0:TCOL], tA[:])

            def a_part1_shard(li, table, s):
                hTs = slabAp.tile([128, SPLIT], f32r, tag="s1")
                nc.sync.dma_start(hTs[:], hT_ag1[s * 128:(s + 1) * 128])
                for j in range(JSPLIT):
                    a_block(li, table, hTs, 0, s, j)

            def a_part2_shard(li, table, s):
                hTs = slabAp.tile([128, NPC - SPLIT], f32r, tag="s2")
                nc.sync.dma_start(hTs[:], hT_ag2[s * 128:(s + 1) * 128])
                for j in range(JSPLIT, CPB):
                    a_block(li, table, hTs, SPLIT, s, j)

            # ---- layer 0 prologue: xh_own + full table from xT_stack
            for j in range(CPB):
                emit_xh_own_j(0, j, hT_buf[0])
            table = dramp.tile([NROWS, ROW], f32r, tag="tab", name="table_l0")
            for s in range(NSH):
                hTs = slabAp.tile([128, SPLIT], f32r, tag="s1")
                nc.sync.dma_start(
                    hTs[:], xT_stack.ap()[s * 128:(s + 1) * 128, 0:SPLIT])
                for j in range(JSPLIT):
                    a_block(0, table, hTs, 0, s, j)
            for s in range(NSH):
                hTs = slabAp.tile([128, NPC - SPLIT], f32r, tag="s2")
                nc.sync.dma_start(
                    hTs[:], xT_stack.ap()[s * 128:(s + 1) * 128, SPLIT:NPC])
                for j in range(JSPLIT, CPB):
                    a_block(0, table, hTs, SPLIT, s, j)

            for li in range(NL):
                bias_l = bias_sb[:, li * 128:(li + 1) * 128]
                last = li == NL - 1
                hout = hT_buf[(li + 1) % 2]
                xho = xho_buf[li % 2]
                if not last:
                    next_table = dramp.tile([NROWS, ROW], f32r, tag="tab",
                                            name=f"table_l{li + 1}")

                # ---- phase B: per dst block
                tabW = [table[wb:wb + HALF] for wb in cfg.wbase]
                qn = 0
                for b in range(CPB):
                    tws = [int(TW[w, b]) for w in range(NWIN)]
                    tt = sum(tws)
                    o = seg_off[b]
                    g1 = g1p.tile([128, TTCAP, ROW], f32r, name="g1")
                    so = 0
                    for w in range(NWIN):
                        tw = tws[w]
                        if tw == 0:
                            continue
                        nc.gpsimd.dma_gather(
                            out_ap=g1[:, so:so + tw, :], in_ap=tabW[w],
                            idxs_ap=idx_sb[:, (o + so) * 8:(o + so + tw) * 8],
                            num_idxs=128 * tw, num_idxs_reg=regs[tw],
                            elem_size=ROW, single_packet=False,
                            queue_num=qn)
                        qn = (qn + 1) % 4
                        so += tw
                    # attention: exm = mask * exp(lrelu(es_src + ed_dst))
                    tat = attp.tile([128, TTCAP, 2], f32, tag="tat")
                    nc.vector.tensor_tensor(
                        out=tat[:, 0:tt, :], in0=g1[:, 0:tt, 256:258],
                        in1=xho[:, b:b + 1, 258:260].broadcast_to(
                            (128, tt, 2)),
                        op=ALU.add)
                    lk = attp.tile([128, TTCAP, 2], f32, tag="lk")
                    nc.vector.tensor_scalar(
                        out=lk[:, 0:tt, :], in0=tat[:, 0:tt, :],
                        scalar1=0.2, scalar2=None, op0=ALU.mult)
                    nc.vector.tensor_tensor(
                        out=lk[:, 0:tt, :], in0=lk[:, 0:tt, :],
                        in1=tat[:, 0:tt, :], op=ALU.max)
                    exm = attp.tile([128, TTCAP, 2], f32, tag="exm")
                    nc.scalar.activation(exm[:, 0:tt, :], lk[:, 0:tt, :],
                                         ACTF.Exp)
                    nc.vector.tensor_tensor(
                        out=exm[:, 0:tt, :], in0=exm[:, 0:tt, :],
                        in1=mask_sb[:, o:o + tt, :], op=ALU.mult)
                    # self-loop attention (own node, per partition)
                    tat_s = attp.tile([128, 2], f32, tag="tats")
                    nc.vector.tensor_tensor(
                        out=tat_s[:], in0=xho[:, b, 256:258],
                        in1=xho[:, b, 258:260], op=ALU.add)
                    lk_s = attp.tile([128, 2], f32, tag="lks")
                    nc.vector.tensor_scalar(
                        out=lk_s[:], in0=tat_s[:], scalar1=0.2,
                        scalar2=None, op0=ALU.mult)
                    nc.vector.tensor_tensor(
                        out=lk_s[:], in0=lk_s[:], in1=tat_s[:], op=ALU.max)
                    ex_s = attp.tile([128, 2], f32, tag="exs")
                    nc.scalar.activation(ex_s[:], lk_s[:], ACTF.Exp)
                    # msg in-place in g1: cols 0:256 *= exm (per tile-slot;
                    # head 0 on DVE at 4x, head 1 on ACT), cols 256:258 = exm
                    for t in range(tt):
                        nc.vector.tensor_scalar(
                            out=g1[:, t, 0:128], in0=g1[:, t, 0:128],
                            scalar1=exm[:, t, 0:1], scalar2=None,
                            op0=ALU.mult)
                        nc.scalar.activation(
                            g1[:, t, 128:256], g1[:, t, 128:256],
                            ACTF.Copy, scale=exm[:, t, 1:2])
                    nc.vector.tensor_copy(g1[:, 0:tt, 256:258],
                                          exm[:, 0:tt, :])
                    psum = psumBp.tile([128, 258], f32)
                    for t in range(tt):
                        nc.tensor.matmul(
                            psum[:], identb_sb[:], g1[:, t, 0:258],
                            start=(t == 0), stop=(t == tt - 1))
                    # epilogue with self-loop fold
                    den = epp.tile([128, 2], f32, tag="den")
                    nc.vector.tensor_tensor(
                        out=den[:], in0=psum[:, 256:258], in1=ex_s[:],
                        op=ALU.add)
                    rec = epp.tile([128, 2], f32, tag="rec")
                    nc.vector.reciprocal(rec[:], den[:])
                    h_blk = epp.tile([128, 128], f32, tag="hblk")
                    m1 = epp.tile([128, 128], f32, tag="m1")
                    for hh, dst in ((0, h_blk), (1, m1)):
                        sf = epp.tile([128, 128], f32, tag=f"sf{hh}")
                        nc.vector.tensor_scalar(
                            out=sf[:], in0=xho[:, b, hh * 128:(hh + 1) * 128],
                            scalar1=ex_s[:, hh:hh + 1], scalar2=None,
                            op0=ALU.mult)
                        nc.vector.tensor_tensor(
                            out=sf[:], in0=sf[:],
                            in1=psum[:, hh * 128:(hh + 1) * 128], op=ALU.add)
                        nc.vector.tensor_scalar(
                            out=dst[:], in0=sf[:],
                            scalar1=rec[:, hh:hh + 1], scalar2=0.5,
                            op0=ALU.mult, op1=ALU.mult)
                    nc.vector.tensor_tensor(
                        out=h_blk[:], in0=h_blk[:], in1=m1[:], op=ALU.add)
                    nc.vector.tensor_tensor(
                        out=h_blk[:], in0=h_blk[:], in1=bias_l, op=ALU.add)
                    if not last:
                        # ELU = (max(x,0)-1) + exp(min(x,0))
                        mn = epp.tile([128, 128], f32, tag="mn")
                        nc.vector.tensor_scalar(
                            out=mn[:], in0=h_blk[:], scalar1=0.0,
                            scalar2=None, op0=ALU.min)
                        emn = epp.tile([128, 128], f32, tag="emn")
                        nc.scalar.activation(emn[:], mn[:], ACTF.Exp)
                        nc.vector.tensor_scalar(
                            out=h_blk[:], in0=h_blk[:], scalar1=0.0,
                            scalar2=-1.0, op0=ALU.max, op1=ALU.add)
                        nc.vector.tensor_tensor(
                            out=h_blk[:], in0=h_blk[:], in1=emn[:],
                            op=ALU.add)
                        psT = psumTp.tile([128, 128], f32)
                        nc.tensor.transpose(psT[:], h_blk[:], ident_sb[:])
                        nc.vector.tensor_copy(
                            hout[:, b * 128:(b + 1) * 128], psT[:])
                        # next layer's own-row transform for this block
                        emit_xh_own_j(li + 1, b, hout)
                        if b == JSPLIT - 1:
                            # first part of hout done: overlap its AllGather
                            hT_loc1 = dramp.tile([128, SPLIT], f32r,
                                                 tag="hloc1")
                            nc.sync.dma_start(hT_loc1[:], hout[:, 0:SPLIT])
                            hT_ag1 = dramp.tile([NSH * 128, SPLIT], f32r,
                                                tag="hag1",
                                                addr_space="Shared")
                            nc.gpsimd.collective_compute(
                                "AllGather", ALU.bypass,
                                replica_groups=[list(range(cfg.ncores))],
                                ins=[hT_loc1.opt()], outs=[hT_ag1.opt()])
                        # interleave next layer's phase A (part 1) so it
                        # executes during this B phase
                        if JSPLIT + 3 <= b < JSPLIT + 3 + NSH:
                            a_part1_shard(li + 1, next_table, b - JSPLIT - 3)
                    else:
                        nc.sync.dma_start(
                            out[b * 128:(b + 1) * 128, :], h_blk[:])
                if not last:
                    hT_loc2 = dramp.tile([128, NPC - SPLIT], f32r,
                                         tag="hloc2")
                    nc.sync.dma_start(hT_loc2[:], hout[:, SPLIT:NPC])
                    hT_ag2 = dramp.tile([NSH * 128, NPC - SPLIT], f32r,
                                        tag="hag2", addr_space="Shared")
                    nc.gpsimd.collective_compute(
                        "AllGather", ALU.bypass,
                        replica_groups=[list(range(cfg.ncores))],
                        ins=[hT_loc2.opt()], outs=[hT_ag2.opt()])
                    for s in range(NSH):
                        a_part2_shard(li + 1, next_table, s)
                    table = next_table
    nc.compile()
    return nc


# ------------------------------------------------------------------ driver

def in_map(pc):
    return dict(xT_stack=pc["xT_stack"], xT_local=pc["xT_local"],
                w_ext=pc["w_ext"], bias=pc["bias"], ident=pc["ident"],
                identb=pc["identb"], idxw=pc["idxw"], maskw=pc["maskw"])


def run(cfg, x, edge_index, params, trace=False):
    from concourse.bass_utils import run_bass_kernel_spmd
    pp, per_core = host_arrays(cfg, x, edge_index, params)
    nc = build_nc(cfg, pp)
    in_maps = [in_map(pc) for pc in per_core]
    res = run_bass_kernel_spmd(
        nc, in_maps, core_ids=list(range(cfg.ncores)), trace=trace)
    full = np.concatenate([res.results[c]["out"] for c in range(cfg.ncores)])
    return full[pp["perm"]], res


# ------------------------------------------------------------- entry point

_CFG = Cfg()


def kernel(x, edge_index, W0, a_src0, a_dst0, b0, W1, a_src1, a_dst1, b1,
           W2, a_src2, a_dst2, b2):
    """Full-input GAT kernel: shards across 8 NeuronCores internally."""
    params = [(W0, a_src0, a_dst0, b0), (W1, a_src1, a_dst1, b1),
              (W2, a_src2, a_dst2, b2)]
    out, _ = run(_CFG, x, edge_index, params, trace=False)
    return np.asarray(out, dtype=np.float32)
